# revision 1
# baseline (speedup 1.0000x reference)
"""DispNetC correlation volume on 8 NeuronCores (Trainium2, Bass/Tile).

out[b, d, h, w] = mean_c(L[b,c,h,w] * R[b,c,h,w-d]), d in [0,40), 0 where w<d.

Sharding: data-parallel over batch (B=8 -> 1 sample per core). Per core:

1. Load L, R into SBUF as [c_lo(128 part), (c_hi, h, w)] in NSPLIT chunks.
2. Per h: 2 accumulating fp32 matmuls -> PSUM Gram G[w, w'] = sum_c L[c,w]R[c,w'].
   The needed band is out[d, w] = G[w, w-d]/256 - 40 diagonals, which are
   partition-coupled in [w, w'] layout (no legal AP reads a diagonal).
3. Re-layout to h-on-partition form via one DRAM round trip:
   DVE-copy Grams into G_all[w, h*128 + w'], one DMA G_all -> scratch[w, h, w']
   (+GUARD prefix for w' < 0 reads), two DMAs back as
   X[p = 64q + h, i*103 + j] holding G[w = 64q + i, w' = 64q - 39 + j]
   (clipped to valid w'; X is pre-zeroed so w' < 0 holes = the w<d zeros).
4. In X a diagonal d for ALL h is an uncoupled strided AP: element
   (p, i, j=i+39-d) sits at free offset 104*i + 39 - d ->
   one DVE scalar-mul (x 1/256) per d -> O[p, 64*d + i].
5. Two DMAs (one per q) write O to out[d, h, w] in 512B-contiguous runs.

DMA count is minimized aggressively: this environment shows ~14us fixed cost
per DMA instruction, dominating everything else.
"""

import numpy as np

C, H, W, D = 256, 64, 128, 40
NS = 103                 # per-q window width (39 + 64)
XF = 64 * NS             # X free size
F3 = D * 64              # O free size
GUARD = 64               # scratch guard elems for w' < 0 reads
N_CORES = 8
NSPLIT = 2               # input load chunks per tensor (h-blocks)

_cache = {}


def _build(n_cores=N_CORES, nsplit=NSPLIT):
    import concourse.bass as bass
    import concourse.bacc as bacc
    import concourse.mybir as mybir
    from concourse.tile import TileContext

    f32 = mybir.dt.float32
    nc = bacc.Bacc("TRN2", target_bir_lowering=False, debug=False,
                   num_devices=n_cores)
    l_in = nc.dram_tensor("l", [C, H, W], f32, kind="ExternalInput")
    r_in = nc.dram_tensor("r", [C, H, W], f32, kind="ExternalInput")
    out = nc.dram_tensor("out", [D, H, W], f32, kind="ExternalOutput")

    HBLK = H // nsplit

    with TileContext(nc) as tc:
        with (
            tc.tile_pool(name="inp", bufs=2) as inp,
            tc.tile_pool(name="fix", bufs=1) as fix,
            tc.tile_pool(name="ps", bufs=6, space="PSUM") as psp,
            tc.tile_pool(name="dram", bufs=1, space="DRAM") as dp,
        ):
            g_all = fix.tile([128, H * W], f32, tag="gall")
            ga3 = g_all[:, :].rearrange("w (h x) -> w h x", x=W)
            x_t = fix.tile([128, XF], f32, tag="x")
            o_t = fix.tile([128, F3], f32, tag="o")
            scratch = dp.tile([GUARD + 128 * H * W], f32)
            sflat = scratch[:]

            lv = l_in.ap().rearrange("(ch p) h w -> p ch h w", ch=2)
            rv = r_in.ap().rearrange("(ch p) h w -> p ch h w", ch=2)

            for blk in range(nsplit):
                h0 = blk * HBLK
                lt = inp.tile([128, 2 * HBLK * W], f32, tag="lt")
                rt = inp.tile([128, 2 * HBLK * W], f32, tag="rt")
                lt4 = lt[:, :].rearrange("p (ch h w) -> p ch h w", ch=2, h=HBLK)
                rt4 = rt[:, :].rearrange("p (ch h w) -> p ch h w", ch=2, h=HBLK)
                nc.sync.dma_start(lt4, lv[:, :, h0 : h0 + HBLK, :])
                nc.scalar.dma_start(rt4, rv[:, :, h0 : h0 + HBLK, :])
                for hb in range(HBLK):
                    h = h0 + hb
                    gm = psp.tile([128, W], f32, tag="gram")
                    for ch in range(2):
                        nc.tensor.matmul(
                            gm[:, :], lt4[:, ch, hb, :], rt4[:, ch, hb, :],
                            start=(ch == 0), stop=(ch == 1),
                        )
                    nc.vector.tensor_copy(ga3[:, h, :], gm[:, :])

            # G_all -> DRAM scratch (one DMA): scratch[GUARD + w*H*W + h*W + w']
            sc3 = sflat[GUARD:].rearrange("(w h x) -> w h x", w=128, h=H)
            nc.sync.dma_start(sc3, ga3)

            # baseline-zero X (covers the q=0 j<39 hole = w<d zeros, and
            # keeps CoreSim's interval-based init tracking happy)
            nc.vector.memset(x_t[:, :], 0.0)

            # readback per q: X[64q+h, i*103+j] <- scratch[w=64q+i, h, w']
            sc4 = sflat[GUARD:].rearrange("(i h x) -> i h x", i=128, h=H)
            for q in range(2):
                j0 = 39 if q == 0 else 0
                wlo = 64 * q - 39 + j0
                src_ap = sc4[64 * q : 64 * q + 64, :, wlo : wlo + NS - j0]
                dst = x_t[64 * q : 64 * q + 64, :].rearrange(
                    "h (i j) -> h i j", j=NS)[:, :, j0:]
                eng = nc.sync if q == 0 else nc.scalar
                eng.dma_start(dst, src_ap.transpose([1, 0, 2]))

            # per-diagonal extraction with 1/C scale
            xs = x_t[:, :]
            ovw = o_t[:, :].rearrange("p (d i) -> p d i", d=D)
            for d in range(D):
                lo = 39 - d
                nc.vector.tensor_scalar_mul(
                    ovw[:, d, :],
                    xs[:, lo : lo + 104 * 63 + 1 : 104],
                    1.0 / C,
                )

            # out DMAs: one per q, contiguous partitions [64q, 64q+64)
            dstq = out.ap().rearrange("d h (two w) -> two h d w", two=2)
            for q in range(2):
                srcq = o_t[64 * q : 64 * q + 64, :].rearrange(
                    "h (d w) -> h d w", d=D)
                eng = nc.sync if q == 0 else nc.scalar
                eng.dma_start(dstq[q], srcq)

    nc.compile()
    return nc


def _get_program():
    if "nc" not in _cache:
        _cache["nc"] = _build()
    return _cache["nc"]


def kernel(conv3a_l: np.ndarray, conv3a_r: np.ndarray) -> np.ndarray:
    from concourse import bass_utils

    nc = _get_program()
    conv3a_l = np.ascontiguousarray(conv3a_l, dtype=np.float32)
    conv3a_r = np.ascontiguousarray(conv3a_r, dtype=np.float32)
    in_maps = [
        {"l": conv3a_l[b], "r": conv3a_r[b]} for b in range(N_CORES)
    ]
    res = bass_utils.run_bass_kernel_spmd(nc, in_maps,
                                          core_ids=list(range(N_CORES)))
    return np.stack([res.results[b]["out"] for b in range(N_CORES)], axis=0)



# revision 2
# speedup vs baseline: 4.4230x; 4.4230x over previous
"""DispNetC correlation volume on 8 NeuronCores (Trainium2, Bass/Tile).

out[b, d, h, w] = mean_c(L[b,c,h,w] * R[b,c,h,w-d]), d in [0,40), 0 where w<d.
Data-parallel over batch (B=8 -> 1 sample per core).

Environment facts (measured): ~208us fixed dispatch overhead per execution
(additive); HBM ~87 GB/s per core; DMA cost ~= max(bytes/BW, n_descriptors *
~33ns) per queue; sub-512B write runs catastrophically slow; SWDGE (gpsimd)
~5x slower for bulk; SBUF APs cannot cross partitions in DMA (no sb2sb
transpose).

Per-core pipeline (h split into NCHUNK chunks for load/compute overlap):
1. Load L, R chunks f32 -> SBUF [c_lo(128), (c_hi, h, w)] (HWDGE, 8KB runs).
2. Per h: 2 accumulating fp32 matmuls -> PSUM Gram G[w, w'] (4 h per bank).
3. Per 4-h group: scaled copy (x 1/C) + cast -> bf16 g_all2[w, (h, 167-row)]
   where each row = 39 zeros ++ G[w, h, 0:128]. (DVE/ACT alternating.)
4. Dump g_all2 chunk -> DRAM scratch, contiguous per partition (5.3KB runs).
5. Readback the diagonal band directly: scratch viewed with row pitch
   P+1=10689 makes offset(w, h, dr) = w*10689 + h*167 + dr linear
   (dr = 39-d); reads where w < d land exactly on the 39 zero-pad columns.
   dst o_pre[h2-part, (hp, w, dr)] bf16 - 80B runs, the descriptor-bound
   step, split across queues/chunks.
6. DVE/ACT reshuffle+cast: o_pre -> o_t[h2, (d, hp, w)] f32.
7. Out DMA per chunk: [h2, (d, hp, w)] -> out[d, h, w], 1KB runs.
"""

import numpy as np

C, H, W, D = 256, 64, 128, 40
PAD = 39                  # zero columns per g row (covers w' = w-d < 0)
ROW = W + PAD             # 167
GP = H * ROW              # 10688 g_all2 free elems (write pitch)
RP = GP + 1               # 10689 readback row pitch
N_CORES = 8
NCHUNK = 4                # h chunks
HC = H // NCHUNK          # 16 h per chunk
RB_ENGINE = "hwdge"       # "hwdge" | "gpsimd" for the band readback

_cache = {}


def _build(rb_engine=RB_ENGINE, nchunk=NCHUNK):
    import concourse.bass as bass
    import concourse.bacc as bacc
    import concourse.mybir as mybir
    from concourse.tile import TileContext

    f32 = mybir.dt.float32
    bf16 = mybir.dt.bfloat16
    hc = H // nchunk
    nc = bacc.Bacc("TRN2", target_bir_lowering=False, debug=False,
                   num_devices=N_CORES)
    l_in = nc.dram_tensor("l", [C, H, W], f32, kind="ExternalInput")
    r_in = nc.dram_tensor("r", [C, H, W], f32, kind="ExternalInput")
    out = nc.dram_tensor("out", [D, H, W], f32, kind="ExternalOutput")

    lv = l_in.ap().rearrange("(ch p) h w -> p ch h w", ch=2)
    rv = r_in.ap().rearrange("(ch p) h w -> p ch h w", ch=2)

    with TileContext(nc) as tc:
        with (
            tc.tile_pool(name="inp", bufs=2 if nchunk >= 4 else 1) as inp,
            tc.tile_pool(name="fix", bufs=1) as fix,
            tc.tile_pool(name="ps", bufs=1, space="PSUM") as psp,
            tc.tile_pool(name="dram", bufs=1, space="DRAM") as dp,
        ):
            g_all = fix.tile([128, GP], bf16, tag="gall")
            gv = g_all[:, :].rearrange("w (h k) -> w h k", k=ROW)
            o_pre = fix.tile([32, 2 * 128 * D], bf16, tag="opre")
            op4 = o_pre[:, :].rearrange("p (hp w dr) -> p hp w dr",
                                        hp=2, w=128)
            o_t = fix.tile([32, D * 2 * 128], f32, tag="ot")
            ov4 = o_t[:, :].rearrange("p (d hp w) -> p d hp w", d=D, hp=2)

            # per-chunk scratch: write pitch CGP, readback pitch CGP+1
            CGP = hc * ROW

            # zero pad columns once (covers w < d region of the band)
            nc.vector.memset(g_all[:, :], 0.0)

            out4 = out.ap().rearrange("d (h2 hp) w -> h2 d hp w", hp=2)

            for j in range(nchunk):
                h0 = j * hc
                lt = inp.tile([128, 2 * hc * W], f32, tag="lt")
                rt = inp.tile([128, 2 * hc * W], f32, tag="rt")
                lt4 = lt[:, :].rearrange("p (ch h w) -> p ch h w", ch=2, h=hc)
                rt4 = rt[:, :].rearrange("p (ch h w) -> p ch h w", ch=2, h=hc)
                nc.sync.dma_start(lt4, lv[:, :, h0:h0 + hc, :])
                nc.scalar.dma_start(rt4, rv[:, :, h0:h0 + hc, :])

                # Grams: 4 h per PSUM bank
                for g in range(hc // 4):
                    gm = psp.tile([128, 512], f32, tag=f"g{g % 4}")
                    for k in range(4):
                        hb = g * 4 + k
                        for ch in range(2):
                            nc.tensor.matmul(
                                gm[:, 128 * k:128 * (k + 1)],
                                lt4[:, ch, hb, :], rt4[:, ch, hb, :],
                                start=(ch == 0), stop=(ch == 1),
                            )
                    dst = gv[:, h0 + 4 * g:h0 + 4 * g + 4, PAD:ROW]
                    src = gm[:, :].rearrange("p (h w) -> p h w", h=4)
                    if g % 2 == 0:
                        nc.vector.tensor_scalar_mul(dst, src, 1.0 / C)
                    else:
                        nc.scalar.activation(
                            dst, src, mybir.ActivationFunctionType.Copy,
                            scale=1.0 / C)

                # dump this chunk's rows to scratch (per-partition contiguous)
                scratch = dp.tile([128 * (CGP + 1)], bf16, tag=f"sc{j}")
                sflat = scratch[:]
                wview = sflat[0:128 * CGP].rearrange("(w f) -> w f", w=128)
                eng = nc.sync if j % 2 == 0 else nc.scalar
                eng.dma_start(wview, g_all[:, h0 * ROW:(h0 + hc) * ROW])

                # band readback: o_pre[h2, hp, w, dr] <- scratch with row
                # pitch CGP+1: offset = w*(CGP+1) + hloc*167 + dr lands on
                # stored G[w, hloc, w-d] (or the zero pad when w < d).
                rview = sflat[:].rearrange("(w f) -> w f", w=128)
                rv5 = rview[:, 0:CGP].rearrange(
                    "w (h2 hp k) -> w h2 hp k", hp=2, k=ROW)
                p0 = j * (hc // 2)
                for q in range(2):
                    for hp in range(2):
                        src = rv5[64 * q:64 * q + 64, :, hp, 0:D]
                        # src axes (i, h2, dr) -> (h2, i, dr)
                        src = src.transpose([1, 0, 2])
                        dst = op4[p0:p0 + hc // 2, hp,
                                  64 * q:64 * q + 64, :]
                        if rb_engine == "gpsimd":
                            nc.gpsimd.dma_start(dst, src)
                        else:
                            eng = nc.sync if q == 0 else nc.scalar
                            eng.dma_start(dst, src)

            # reshuffle+cast to f32 (d-major), split across DVE/ACT
            # (compute engines require base partition 0 - do it once,
            # after all chunk readbacks land)
            src_all = op4[:, :, :, ::-1].transpose([0, 3, 1, 2])
            nc.vector.tensor_copy(ov4[:, 0:D // 2], src_all[:, 0:D // 2])
            nc.scalar.activation(
                ov4[:, D // 2:D], src_all[:, D // 2:D],
                mybir.ActivationFunctionType.Copy)

            # out: [h2, (d, hp, w)] -> out[d, h, w], 1KB runs, d-halves
            # split across the two HWDGE queues
            nc.sync.dma_start(out4[:, 0:D // 2], ov4[:, 0:D // 2])
            nc.scalar.dma_start(out4[:, D // 2:D], ov4[:, D // 2:D])

    nc.compile()
    return nc


def _get_program():
    if "nc" not in _cache:
        _cache["nc"] = _build()
    return _cache["nc"]


def kernel(conv3a_l: np.ndarray, conv3a_r: np.ndarray) -> np.ndarray:
    from concourse import bass_utils

    nc = _get_program()
    conv3a_l = np.ascontiguousarray(conv3a_l, dtype=np.float32)
    conv3a_r = np.ascontiguousarray(conv3a_r, dtype=np.float32)
    in_maps = [
        {"l": conv3a_l[b], "r": conv3a_r[b]} for b in range(N_CORES)
    ]
    res = bass_utils.run_bass_kernel_spmd(nc, in_maps,
                                          core_ids=list(range(N_CORES)))
    return np.stack([res.results[b]["out"] for b in range(N_CORES)], axis=0)
